# revision 2
# baseline (speedup 1.0000x reference)
"""AttentionGate (3D, 1x1x1 convs) as a data-parallel Bass kernel on 8 TRN2 NeuronCores.

Math (per voxel v, channels c):
    a   = relu(Wg @ g_v + Wx @ x_v)        # [CI]
    psi = sigmoid(Wpsi @ a)                # scalar
    out_v = x_v * psi                      # [CF]

Fusions used:
  - Wg@g + Wx@x == [Wx|Wg] @ [x;g]  -> one K=128 matmul per tile.
  - relu rides the PSUM->SBUF copy on ScalarE.
  - psi broadcast across the 64 x-channels via a K=1 matmul with a ones vector.
  - final gate is one VectorE multiply.

Sharding: depth D=64 split 8 ways (pure data parallel, weights replicated).
"""

import os
import sys

for _p in ("/opt/trn_rl_repo", "/root/.axon_site/_ro/trn_rl_repo"):
    if os.path.isdir(_p) and _p not in sys.path:
        sys.path.append(_p)

import numpy as np

import concourse.bass as bass
import concourse.tile as tile
from concourse import bacc, mybir
from concourse.bass_utils import run_bass_kernel_spmd

B = 2
CF = 64   # x channels
CG = 64   # g channels
CI = 32   # intermediate channels
D = H = W = 64
N_CORES = 8
D_LOC = D // N_CORES          # 8 depth slices per core
HW = H * W                    # 4096
N_SLABS = B * D_LOC           # 16 slabs of [C, HW] per core
CHUNK = 512                   # fp32 matmul moving-operand max
N_CHUNKS = HW // CHUNK        # 8

FP32 = mybir.dt.float32


def build_nc():
    nc = bacc.Bacc("TRN2", target_bir_lowering=False, debug=False,
                   num_devices=N_CORES)

    x_ext = nc.dram_tensor("x", [B, CF, D_LOC, HW], FP32, kind="ExternalInput")
    g_ext = nc.dram_tensor("g", [B, CG, D_LOC, HW], FP32, kind="ExternalInput")
    # [Wx|Wg]^T, shape [CF+CG, CI] = [128, 32]; host precomputes the transpose.
    wcat_ext = nc.dram_tensor("wcat_t", [CF + CG, CI], FP32, kind="ExternalInput")
    # Wpsi^T, shape [CI, 1]
    wpsi_ext = nc.dram_tensor("wpsi_t", [CI, 1], FP32, kind="ExternalInput")
    out_ext = nc.dram_tensor("out", [B, CF, D_LOC, HW], FP32, kind="ExternalOutput")

    with tile.TileContext(nc) as tc:
        with (
            tc.tile_pool(name="weights", bufs=1) as wpool,
            tc.tile_pool(name="gx", bufs=3) as gx_pool,
            tc.tile_pool(name="outp", bufs=3) as out_pool,
            tc.tile_pool(name="small", bufs=4) as small_pool,
            tc.tile_pool(name="ps_a", bufs=2, space="PSUM") as ps_a_pool,
            tc.tile_pool(name="ps_psi", bufs=2, space="PSUM") as ps_psi_pool,
        ):
            wcat_sb = wpool.tile([CF + CG, CI], FP32)
            nc.sync.dma_start(wcat_sb[:], wcat_ext[:])
            wpsi_sb = wpool.tile([CI, 1], FP32)
            nc.sync.dma_start(wpsi_sb[:], wpsi_ext[:])
            ones_sb = wpool.tile([1, CF], FP32)
            nc.vector.memset(ones_sb[:], 1.0)

            for b in range(B):
                for d in range(D_LOC):
                    gx = gx_pool.tile([CF + CG, HW], FP32)
                    nc.sync.dma_start(gx[0:CF, :], x_ext[b, :, d, :])
                    nc.sync.dma_start(gx[CF:CF + CG, :], g_ext[b, :, d, :])
                    out_sb = out_pool.tile([CF, HW], FP32)

                    for j in range(N_CHUNKS):
                        cs = slice(j * CHUNK, (j + 1) * CHUNK)
                        ps_a = ps_a_pool.tile([CI, CHUNK], FP32)
                        nc.tensor.matmul(ps_a[:], wcat_sb[:], gx[:, cs],
                                         start=True, stop=True)
                        a_sb = small_pool.tile([CI, CHUNK], FP32, tag="a")
                        nc.scalar.activation(a_sb[:], ps_a[:],
                                             mybir.ActivationFunctionType.Relu)
                        ps_psi = ps_psi_pool.tile([1, CHUNK], FP32)
                        nc.tensor.matmul(ps_psi[:], wpsi_sb[:], a_sb[:],
                                         start=True, stop=True)
                        sig_sb = small_pool.tile([1, CHUNK], FP32, tag="sig")
                        nc.scalar.activation(sig_sb[:], ps_psi[:],
                                             mybir.ActivationFunctionType.Sigmoid)
                        ps_b = ps_a_pool.tile([CF, CHUNK], FP32, tag="ps_b")
                        nc.tensor.matmul(ps_b[:], ones_sb[:], sig_sb[:],
                                         start=True, stop=True)
                        nc.vector.tensor_mul(out_sb[:, cs], gx[0:CF, cs], ps_b[:])

                    nc.sync.dma_start(out_ext[b, :, d, :], out_sb[:])

    nc.compile()
    return nc


_NC = None


def _get_nc():
    global _NC
    if _NC is None:
        _NC = build_nc()
    return _NC


def _shard_inputs(x, g, Wg, Wx, Wpsi):
    wcat_t = np.ascontiguousarray(np.concatenate([Wx, Wg], axis=1).T,
                                  dtype=np.float32)
    wpsi_t = np.ascontiguousarray(Wpsi.T, dtype=np.float32)
    in_maps = []
    for i in range(N_CORES):
        dsl = slice(i * D_LOC, (i + 1) * D_LOC)
        in_maps.append({
            "x": np.ascontiguousarray(x[:, :, dsl]).reshape(B, CF, D_LOC, HW),
            "g": np.ascontiguousarray(g[:, :, dsl]).reshape(B, CG, D_LOC, HW),
            "wcat_t": wcat_t,
            "wpsi_t": wpsi_t,
        })
    return in_maps


def run(inputs, trace=False):
    """Run on hardware; returns (full_output, BassKernelResults)."""
    nc = _get_nc()
    in_maps = _shard_inputs(**inputs)
    res = run_bass_kernel_spmd(nc, in_maps, list(range(N_CORES)), trace=trace)
    shards = [res.results[i]["out"].reshape(B, CF, D_LOC, H, W)
              for i in range(N_CORES)]
    full = np.concatenate(shards, axis=2)
    return full, res


def kernel(**inputs) -> np.ndarray:
    full, _ = run(inputs, trace=False)
    return full


# revision 4
# speedup vs baseline: 2.7527x; 2.7527x over previous
"""AttentionGate (3D, 1x1x1 convs) as a data-parallel Bass kernel on 8 TRN2 NeuronCores.

Math (per voxel v):
    a     = relu(Wg @ g_v + Wx @ x_v)      # [CI]
    psi_v = sigmoid(Wpsi @ a)              # scalar
    out_v = x_v * psi_v                    # [CF]

Structure per 512-voxel chunk (per core):
  - MM1 (f32): [Wx|Wg]^T stacked-channel matmul, K=128 -> [CI, 512] PSUM.
    (Wg@g + Wx@x == [Wx|Wg] @ [x;g], one matmul.)
  - ScalarE relu on the PSUM->SBUF copy, output cast to fp16.
  - MM2 (fp16): Wpsi replicated across 64 output columns -> every row of the
    [64, 512] PSUM result is psi_raw; the partition-broadcast is free.
    fp16 streams 1 cycle/row vs f32's 4; only the tiny psi path loses
    mantissa (10-bit), final rel err ~1e-3.
  - ScalarE sigmoid PSUM->SBUF.
  - VectorE multiply with the x half of the stacked input tile (exact f32).

Data layout: the host packs [x_slab; g_slab] contiguously per (b, d) slab so
each input DMA is a single 2 MiB contiguous transfer into all 128 SBUF
partitions (full DMA port utilization, large descriptors).

Sharding: depth D=64 split 8 ways (pure data parallel, weights replicated).
"""

import os
import sys

for _p in ("/opt/trn_rl_repo", "/root/.axon_site/_ro/trn_rl_repo"):
    if os.path.isdir(_p) and _p not in sys.path:
        sys.path.append(_p)

import numpy as np

import concourse.bass as bass
import concourse.tile as tile
from concourse import bacc, mybir
from concourse.bass_utils import run_bass_kernel_spmd

B = 2
CF = 64   # x channels
CG = 64   # g channels
CI = 32   # intermediate channels
D = H = W = 64
N_CORES = 8
D_LOC = D // N_CORES          # 8 depth slices per core
HW = H * W                    # 4096
N_SLABS = B * D_LOC
CHUNK = 512                   # fp32 matmul moving-operand max
N_CHUNKS = HW // CHUNK        # 8

F32 = mybir.dt.float32
F16 = mybir.dt.float16


def build_nc():
    nc = bacc.Bacc("TRN2", target_bir_lowering=False, debug=False,
                   num_devices=N_CORES)

    # [x;g] channel-stacked slabs, host-packed contiguous per (b, d).
    xg_ext = nc.dram_tensor("xg", [B, D_LOC, CF + CG, HW], F32,
                            kind="ExternalInput")
    # [Wx|Wg]^T, shape [CF+CG, CI] = [128, 32]; host precomputes the transpose.
    wcat_ext = nc.dram_tensor("wcat_t", [CF + CG, CI], F32, kind="ExternalInput")
    # Wpsi^T replicated: [CI, CF] = [32, 64] fp16, every column is Wpsi.
    wpsi_ext = nc.dram_tensor("wpsi_rep", [CI, CF], F16, kind="ExternalInput")
    out_ext = nc.dram_tensor("out", [B, D_LOC, CF, HW], F32,
                             kind="ExternalOutput")

    with tile.TileContext(nc) as tc:
        with (
            tc.tile_pool(name="weights", bufs=1) as wpool,
            tc.tile_pool(name="gx", bufs=3) as gx_pool,
            tc.tile_pool(name="outp", bufs=3) as out_pool,
            tc.tile_pool(name="small", bufs=4) as small_pool,
            tc.tile_pool(name="ps_a", bufs=3, space="PSUM") as ps_a_pool,
            tc.tile_pool(name="ps_psi", bufs=3, space="PSUM") as ps_psi_pool,
        ):
            wcat_sb = wpool.tile([CF + CG, CI], F32)
            nc.sync.dma_start(wcat_sb[:], wcat_ext[:])
            wpsi_sb = wpool.tile([CI, CF], F16)
            nc.sync.dma_start(wpsi_sb[:], wpsi_ext[:])

            for b in range(B):
                for d in range(D_LOC):
                    gx = gx_pool.tile([CF + CG, HW], F32)
                    nc.sync.dma_start(gx[:], xg_ext[b, d])
                    out_sb = out_pool.tile([CF, HW], F32)

                    for j in range(N_CHUNKS):
                        cs = slice(j * CHUNK, (j + 1) * CHUNK)
                        ps_a = ps_a_pool.tile([CI, CHUNK], F32)
                        nc.tensor.matmul(ps_a[:], wcat_sb[:], gx[:, cs],
                                         start=True, stop=True)
                        a_sb = small_pool.tile([CI, CHUNK], F16, tag="a")
                        nc.scalar.activation(a_sb[:], ps_a[:],
                                             mybir.ActivationFunctionType.Relu)
                        ps_psi = ps_psi_pool.tile([CF, CHUNK], F32)
                        nc.tensor.matmul(ps_psi[:], wpsi_sb[:], a_sb[:],
                                         start=True, stop=True)
                        sig_sb = small_pool.tile([CF, CHUNK], F32, tag="sig")
                        nc.scalar.activation(sig_sb[:], ps_psi[:],
                                             mybir.ActivationFunctionType.Sigmoid)
                        nc.vector.tensor_mul(out_sb[:, cs], gx[0:CF, cs],
                                             sig_sb[:])

                    nc.sync.dma_start(out_ext[b, d], out_sb[:])

    nc.compile()
    return nc


_NC = None


def _get_nc():
    global _NC
    if _NC is None:
        _NC = build_nc()
    return _NC


def _shard_inputs(x, g, Wg, Wx, Wpsi):
    wcat_t = np.ascontiguousarray(np.concatenate([Wx, Wg], axis=1).T,
                                  dtype=np.float32)
    wpsi_rep = np.ascontiguousarray(
        np.repeat(Wpsi.reshape(CI, 1), CF, axis=1)).astype(np.float16)
    in_maps = []
    for i in range(N_CORES):
        dsl = slice(i * D_LOC, (i + 1) * D_LOC)
        # [B, C, D_LOC, H, W] -> [B, D_LOC, C, HW], x stacked over g
        xs = np.transpose(x[:, :, dsl], (0, 2, 1, 3, 4)).reshape(B, D_LOC, CF, HW)
        gs = np.transpose(g[:, :, dsl], (0, 2, 1, 3, 4)).reshape(B, D_LOC, CG, HW)
        xg = np.ascontiguousarray(np.concatenate([xs, gs], axis=2))
        in_maps.append({
            "xg": xg,
            "wcat_t": wcat_t,
            "wpsi_rep": wpsi_rep,
        })
    return in_maps


def run(inputs, trace=False):
    """Run on hardware; returns (full_output, BassKernelResults)."""
    nc = _get_nc()
    in_maps = _shard_inputs(**inputs)
    res = run_bass_kernel_spmd(nc, in_maps, list(range(N_CORES)), trace=trace)
    # per-core out: [B, D_LOC, CF, HW] -> [B, CF, D_LOC, H, W]
    shards = [
        np.transpose(res.results[i]["out"].reshape(B, D_LOC, CF, H, W),
                     (0, 2, 1, 3, 4))
        for i in range(N_CORES)
    ]
    full = np.concatenate(shards, axis=2)
    return full, res


def kernel(**inputs) -> np.ndarray:
    full, _ = run(inputs, trace=False)
    return full
